# revision 7
# baseline (speedup 1.0000x reference)
"""3x3 valid conv (cross-correlation) of an 8192x8192 fp32 image on 8 TRN2 NeuronCores.

Strategy
--------
Rows of the output (8190) are sharded across 8 cores (1024 rows each; the
last core computes 2 padded garbage rows that are dropped on gather). Each
core receives its input rows WITH the 2-row halo already included, so no
on-device collectives are needed.

Per core, the conv is computed on the TensorEngine as banded matmuls:
for a block of 128 input rows, out[o, c] += sum_p band_d[p, o] * x[p, c+d]
where band_d[p, o] = w[p-o, d] (3 diagonals). The 3 column taps d=0,1,2 are
3 matmuls over column-shifted views of the same SBUF tile, accumulated in
PSUM. 126 output rows are produced per 128-row block.

Precision: the PE's float32r mode runs at full speed (1 cycle/column) but
rounds operands to 12 significand bits. Operands pre-truncated to <=12 bits
pass through exactly (verified on HW), so we split x = x_hi + x_lo and
w = w_hi + w_lo into exact 12-bit pieces and take three product terms
(x_hi*w_hi + x_lo*w_hi + x_hi*w_lo); the dropped x_lo*w_lo term is ~2^-24
relative. End-to-end error lands at the fp32-accumulation envelope (~1e-7).
The x split is done on-chip by the VectorEngine (mantissa mask + subtract),
so HBM traffic stays at 4 bytes/element in + 4 bytes/element out.
"""
import numpy as np

H = 8192
W = 8192
OH = H - 2
OW = W - 2
NCORES = 8
RPC = 1024  # output rows computed per core (core 7: only 1022 valid)
IN_ROWS = RPC + 2  # input rows per core shard
NBLK = 9  # 8 full 126-row blocks + one 16-row rump
BLK_OUT = 126
NSTRIPE = 3  # width stripes per row-block (SBUF pressure)
STRIPE_OW = OW // NSTRIPE  # 2730 output columns per stripe
STRIPE_IN = STRIPE_OW + 2  # 2732 input columns per stripe
WT = 390  # width tile (PSUM free dim, must be even for fp32r); 21 * 390 = 8190
NWT = STRIPE_OW // WT  # 7
MASK = -4096  # 0xFFFFF000: keep sign+exp+11 explicit mantissa bits

_cache = {}


def _build():
    import concourse.bacc as bacc
    import concourse.tile as tile
    import concourse.mybir as mybir

    f32 = mybir.dt.float32
    f32r = mybir.dt.float32r
    i32 = mybir.dt.int32
    nc = bacc.Bacc("TRN2", target_bir_lowering=False, debug=False)
    xs = nc.dram_tensor("xs", [IN_ROWS, W], f32, kind="ExternalInput")
    wb = nc.dram_tensor("wb", [128, 756], f32r, kind="ExternalInput")
    bc = nc.dram_tensor("bc", [128, 1], f32, kind="ExternalInput")
    ys = nc.dram_tensor("ys", [RPC, OW], f32, kind="ExternalOutput")
    with tile.TileContext(nc) as tc:
        with (
            tc.tile_pool(name="wpool", bufs=1) as wpool,
            tc.tile_pool(name="xraw", bufs=2) as xraw,
            tc.tile_pool(name="xhi", bufs=2) as xhi,
            tc.tile_pool(name="xlo", bufs=2) as xlo,
            tc.tile_pool(name="yout", bufs=2) as yout,
            tc.tile_pool(name="psum", bufs=8, space="PSUM") as psum,
        ):
            wt = wpool.tile([128, 756], f32r)
            nc.sync.dma_start(wt[:], wb[:])
            bt = wpool.tile([128, 1], f32)
            nc.sync.dma_start(bt[:], bc[:])
            for j in range(NBLK):
                r0 = j * BLK_OUT
                orows = BLK_OUT if j < NBLK - 1 else RPC - BLK_OUT * (NBLK - 1)
                irows = orows + 2
                for h in range(NSTRIPE):
                    c0 = h * STRIPE_OW
                    xr = xraw.tile([128, STRIPE_IN], f32)
                    nc.sync.dma_start(
                        xr[:irows, :], xs[r0 : r0 + irows, c0 : c0 + STRIPE_IN]
                    )
                    # f32r-typed outputs: DVE rounds x_hi to 12 significand
                    # bits; x_lo = x - x_hi is exact (<=12 bits) so its f32r
                    # rounding is the identity.
                    xh = xhi.tile([128, STRIPE_IN], f32r)
                    xl = xlo.tile([128, STRIPE_IN], f32r)
                    nc.vector.tensor_copy(xh[:irows, :], xr[:irows, :])
                    nc.vector.tensor_sub(xl[:irows, :], xr[:irows, :], xh[:irows, :])
                    yo = yout.tile([126, STRIPE_OW], f32)
                    for t in range(NWT):
                        ps = psum.tile([126, WT], f32)
                        mm = 0
                        for src, base in ((xh, 0), (xl, 0), (xh, 378)):
                            for d in range(3):
                                mm += 1
                                nc.tensor.matmul(
                                    ps[:orows, :],
                                    wt[:irows, base + d * 126 : base + d * 126 + orows],
                                    src[:irows, t * WT + d : t * WT + d + WT],
                                    start=(mm == 1),
                                    stop=(mm == 9),
                                )
                        nc.scalar.activation(
                            yo[:orows, t * WT : (t + 1) * WT],
                            ps[:orows, :],
                            mybir.ActivationFunctionType.Identity,
                            bias=bt[:orows, :],
                            scale=1.0,
                        )
                    nc.sync.dma_start(
                        ys[r0 : r0 + orows, c0 : c0 + STRIPE_OW], yo[:orows, :]
                    )
    nc.compile()
    return nc


def _get_nc():
    if "nc" not in _cache:
        _cache["nc"] = _build()
    return _cache["nc"]


def _trunc12(a):
    """Round fp32 toward zero to 12 significand bits (11 explicit)."""
    return (np.asarray(a, np.float32).view(np.int32) & np.int32(MASK)).view(np.float32)


def make_inputs(x, weight, bias):
    """Host-side shard/prep: per-core input maps for run_bass_kernel_spmd."""
    x = np.asarray(x, np.float32)
    w = np.asarray(weight, np.float32)
    wh = _trunc12(w)
    wl = _trunc12(w - wh)  # w - wh is exact in fp32 and fits 12 bits
    wbm = np.zeros((128, 756), np.float32)
    o = np.arange(BLK_OUT)
    for d in range(3):
        for k in range(3):
            wbm[o + k, d * BLK_OUT + o] = wh[k, d]
            wbm[o + k, 378 + d * BLK_OUT + o] = wl[k, d]
    bcm = np.full((128, 1), np.float32(np.asarray(bias).reshape(-1)[0]), np.float32)
    xpad = np.concatenate(
        [x, np.zeros((NCORES * RPC + 2 - H, W), np.float32)], axis=0
    )
    return [
        {
            "xs": np.ascontiguousarray(xpad[i * RPC : i * RPC + IN_ROWS]),
            "wb": wbm,
            "bc": bcm,
        }
        for i in range(NCORES)
    ]


def kernel(x, weight, bias):
    from concourse.bass_utils import run_bass_kernel_spmd

    nc = _get_nc()
    in_maps = make_inputs(x, weight, bias)
    res = run_bass_kernel_spmd(nc, in_maps, list(range(NCORES)))
    parts = [res.results[i]["ys"] for i in range(NCORES)]
    return np.concatenate(parts, axis=0)[:OH].astype(np.float32)


# revision 8
# speedup vs baseline: 154.4686x; 154.4686x over previous
"""3x3 valid conv (cross-correlation) of an 8192x8192 fp32 image on 8 TRN2 NeuronCores.

Strategy
--------
Rows of the output (8190) are sharded across 8 cores (1024 rows each; the
last core computes 2 padded garbage rows that are dropped on gather). Each
core receives its input rows WITH the 2-row halo already included, so no
on-device collectives are needed.

Per core, the conv is computed on the TensorEngine as banded matmuls:
for a block of 128 input rows, out[o, c] += sum_p band_d[p, o] * x[p, c+d]
where band_d[p, o] = w[p-o, d] (3 diagonals). The 3 column taps d=0,1,2 are
3 matmuls over column-shifted views of the same SBUF tile, accumulated in
PSUM. 126 output rows are produced per 128-row block.

Precision: the PE's float32r mode runs at full speed (1 cycle/column) but
rounds operands to 12 significand bits. Operands pre-truncated to <=12 bits
pass through exactly (verified on HW), so we split x = x_hi + x_lo and
w = w_hi + w_lo into exact 12-bit pieces and take three product terms
(x_hi*w_hi + x_lo*w_hi + x_hi*w_lo); the dropped x_lo*w_lo term is ~2^-24
relative. End-to-end error lands at the fp32-accumulation envelope (~1e-7).
The x split is done on-chip by the VectorEngine (mantissa mask + subtract),
so HBM traffic stays at 4 bytes/element in + 4 bytes/element out.
"""
import numpy as np

H = 8192
W = 8192
OH = H - 2
OW = W - 2
NCORES = 8
RPC = 1024  # output rows computed per core (core 7: only 1022 valid)
IN_ROWS = RPC + 2  # input rows per core shard
NBLK = 9  # 8 full 126-row blocks + one 16-row rump
BLK_OUT = 126
NSTRIPE = 3  # width stripes per row-block (SBUF pressure)
STRIPE_OW = OW // NSTRIPE  # 2730 output columns per stripe
STRIPE_IN = STRIPE_OW + 2  # 2732 input columns per stripe
WT = 390  # width tile (PSUM free dim, must be even for fp32r); 21 * 390 = 8190
NWT = STRIPE_OW // WT  # 7
MASK = -4096  # 0xFFFFF000: keep sign+exp+11 explicit mantissa bits

_cache = {}


def _build(reps=1):
    from contextlib import ExitStack

    import concourse.bacc as bacc
    import concourse.tile as tile
    import concourse.mybir as mybir

    f32 = mybir.dt.float32
    f32r = mybir.dt.float32r
    nc = bacc.Bacc("TRN2", target_bir_lowering=False, debug=False)
    xs = nc.dram_tensor("xs", [IN_ROWS, W], f32, kind="ExternalInput")
    wb = nc.dram_tensor("wb", [128, 756], f32r, kind="ExternalInput")
    bc = nc.dram_tensor("bc", [128, 1], f32, kind="ExternalInput")
    ys = nc.dram_tensor("ys", [RPC, OW], f32, kind="ExternalOutput")
    with tile.TileContext(nc) as tc:
        with (
            tc.tile_pool(name="wpool", bufs=1) as wpool,
            tc.tile_pool(name="xraw", bufs=2) as xraw,
            tc.tile_pool(name="xhi", bufs=2) as xhi,
            tc.tile_pool(name="xlo", bufs=2) as xlo,
            tc.tile_pool(name="yout", bufs=2) as yout,
            tc.tile_pool(name="psum", bufs=8, space="PSUM") as psum,
            ExitStack() as rep_ctx,
        ):
            wt = wpool.tile([128, 756], f32r)
            nc.sync.dma_start(wt[:], wb[:])
            bt = wpool.tile([128, 1], f32)
            nc.sync.dma_start(bt[:], bc[:])
            if reps > 1:
                # timing-only variant: repeat the whole body on-device so
                # per-iteration device time can be isolated from the (large)
                # axon dispatch overhead
                rep_ctx.enter_context(tc.For_i(0, reps, 1))
            for j in range(NBLK):
                r0 = j * BLK_OUT
                orows = BLK_OUT if j < NBLK - 1 else RPC - BLK_OUT * (NBLK - 1)
                irows = orows + 2
                for h in range(NSTRIPE):
                    c0 = h * STRIPE_OW
                    xr = xraw.tile([128, STRIPE_IN], f32)
                    nc.sync.dma_start(
                        xr[:irows, :], xs[r0 : r0 + irows, c0 : c0 + STRIPE_IN]
                    )
                    # f32r-typed outputs: DVE rounds x_hi to 12 significand
                    # bits; x_lo = x - x_hi is exact (<=12 bits) so its f32r
                    # rounding is the identity.
                    xh = xhi.tile([128, STRIPE_IN], f32r)
                    xl = xlo.tile([128, STRIPE_IN], f32r)
                    nc.vector.tensor_copy(xh[:irows, :], xr[:irows, :])
                    nc.vector.tensor_sub(xl[:irows, :], xr[:irows, :], xh[:irows, :])
                    yo = yout.tile([126, STRIPE_OW], f32)
                    for t in range(NWT):
                        ps = psum.tile([126, WT], f32)
                        mm = 0
                        for src, base in ((xh, 0), (xl, 0), (xh, 378)):
                            for d in range(3):
                                mm += 1
                                nc.tensor.matmul(
                                    ps[:orows, :],
                                    wt[:irows, base + d * 126 : base + d * 126 + orows],
                                    src[:irows, t * WT + d : t * WT + d + WT],
                                    start=(mm == 1),
                                    stop=(mm == 9),
                                )
                        nc.scalar.activation(
                            yo[:orows, t * WT : (t + 1) * WT],
                            ps[:orows, :],
                            mybir.ActivationFunctionType.Identity,
                            bias=bt[:orows, :],
                            scale=1.0,
                        )
                    nc.sync.dma_start(
                        ys[r0 : r0 + orows, c0 : c0 + STRIPE_OW], yo[:orows, :]
                    )
    nc.compile()
    return nc


def _get_nc():
    if "nc" not in _cache:
        _cache["nc"] = _build()
    return _cache["nc"]


def _trunc12(a):
    """Round fp32 toward zero to 12 significand bits (11 explicit)."""
    return (np.asarray(a, np.float32).view(np.int32) & np.int32(MASK)).view(np.float32)


def make_inputs(x, weight, bias):
    """Host-side shard/prep: per-core input maps for run_bass_kernel_spmd."""
    x = np.asarray(x, np.float32)
    w = np.asarray(weight, np.float32)
    wh = _trunc12(w)
    wl = _trunc12(w - wh)  # w - wh is exact in fp32 and fits 12 bits
    wbm = np.zeros((128, 756), np.float32)
    o = np.arange(BLK_OUT)
    for d in range(3):
        for k in range(3):
            wbm[o + k, d * BLK_OUT + o] = wh[k, d]
            wbm[o + k, 378 + d * BLK_OUT + o] = wl[k, d]
    bcm = np.full((128, 1), np.float32(np.asarray(bias).reshape(-1)[0]), np.float32)
    xpad = np.concatenate(
        [x, np.zeros((NCORES * RPC + 2 - H, W), np.float32)], axis=0
    )
    return [
        {
            "xs": np.ascontiguousarray(xpad[i * RPC : i * RPC + IN_ROWS]),
            "wb": wbm,
            "bc": bcm,
        }
        for i in range(NCORES)
    ]


def kernel(x, weight, bias):
    from concourse.bass_utils import run_bass_kernel_spmd

    nc = _get_nc()
    in_maps = make_inputs(x, weight, bias)
    res = run_bass_kernel_spmd(nc, in_maps, list(range(NCORES)))
    parts = [res.results[i]["ys"] for i in range(NCORES)]
    return np.concatenate(parts, axis=0)[:OH].astype(np.float32)


# revision 9
# speedup vs baseline: 184.4456x; 1.1941x over previous
"""3x3 valid conv (cross-correlation) of an 8192x8192 fp32 image on 8 TRN2 NeuronCores.

Strategy
--------
Output rows are sharded across 8 cores. Each core computes 8 full 126-row
"band blocks" (1008 rows, out rows [i*1008, i*1008+1008)), and the leftover
126-row slab (out rows 8064..8189) is split BY WIDTH across the cores
(~1024 columns each) so no core runs a mostly-empty rump block. Every core
receives its input rows/cols WITH the 2-element halo already included, so
no on-device collectives are needed.

Per core, the conv runs on the TensorEngine as banded matmuls: for a block
of 128 input rows, out[o, c] += sum_p band_d[p, o] * x[p, c+d] where
band_d[p, o] = w[p-o, d] (3 diagonals). The 3 column taps d=0,1,2 are 3
matmuls over column-shifted views of the same SBUF tile, accumulated in
PSUM. 126 output rows are produced per 128-row block.

Precision: the PE's float32r mode runs at full speed (1 cycle/column) but
rounds both operands to 12 significand bits. Operands pre-rounded to <=12
bits pass through exactly (verified on HW), so we split x = x_hi + x_lo and
w = w_hi + w_lo into exact 12-bit pieces and take three product terms
(x_hi*w_hi + x_lo*w_hi + x_hi*w_lo); the dropped x_lo*w_lo term is ~2^-24
relative. End-to-end error lands at the fp32-accumulation envelope (~2e-7).
The x split is done on-chip by the VectorEngine (an f32->f32r rounding copy
plus an exact subtract), so HBM traffic stays at 4B/element in + 4B/element
out -- the memory roofline of the problem.
"""
import numpy as np

H = 8192
W = 8192
OH = H - 2
OW = W - 2
NCORES = 8
BLK_OUT = 126
NBLK = 8  # full band blocks per core
RPC = NBLK * BLK_OUT  # 1008 contiguous output rows per core
IN_ROWS = RPC + 2  # 1010 input rows per core shard
NSTRIPE = 3  # width stripes per row-block (SBUF pressure)
STRIPE_OW = OW // NSTRIPE  # 2730 output columns per stripe
STRIPE_IN = STRIPE_OW + 2  # 2732 input columns per stripe
WT = 390  # width tile (PSUM free dim, must be even for fp32r); 21 * 390 = 8190
NWT = STRIPE_OW // WT  # 7
# leftover slab: out rows [8064, 8190) split by width across cores
SLAB_R0 = NCORES * RPC  # 8064
SLAB_OC = 1024  # slab output cols per core (core 7: only 1022 valid)
SLAB_IC = SLAB_OC + 2
SLAB_NT = 2  # 2 width tiles of 512
SLAB_WT = 512

_cache = {}


def _build(reps=1):
    from contextlib import ExitStack

    import concourse.bacc as bacc
    import concourse.tile as tile
    import concourse.mybir as mybir

    f32 = mybir.dt.float32
    f32r = mybir.dt.float32r
    nc = bacc.Bacc("TRN2", target_bir_lowering=False, debug=False)
    xs = nc.dram_tensor("xs", [IN_ROWS, W], f32, kind="ExternalInput")
    xs2 = nc.dram_tensor("xs2", [128, SLAB_IC], f32, kind="ExternalInput")
    wb = nc.dram_tensor("wb", [128, 756], f32r, kind="ExternalInput")
    bc = nc.dram_tensor("bc", [128, 1], f32, kind="ExternalInput")
    ys = nc.dram_tensor("ys", [RPC, OW], f32, kind="ExternalOutput")
    ys2 = nc.dram_tensor("ys2", [BLK_OUT, SLAB_OC], f32, kind="ExternalOutput")
    with tile.TileContext(nc) as tc:
        with (
            tc.tile_pool(name="wpool", bufs=1) as wpool,
            tc.tile_pool(name="xraw", bufs=2) as xraw,
            tc.tile_pool(name="xhi", bufs=2) as xhi,
            tc.tile_pool(name="xlo", bufs=2) as xlo,
            tc.tile_pool(name="yout", bufs=2) as yout,
            tc.tile_pool(name="psum", bufs=8, space="PSUM") as psum,
            ExitStack() as rep_ctx,
        ):
            wt = wpool.tile([128, 756], f32r)
            nc.sync.dma_start(wt[:], wb[:])
            bt = wpool.tile([128, 1], f32)
            nc.sync.dma_start(bt[:], bc[:])
            if reps > 1:
                # timing-only variant: repeat the body on-device so per-
                # iteration device time can be isolated from the (large)
                # axon dispatch overhead
                rep_ctx.enter_context(tc.For_i(0, reps, 1))

            def do_stripe(src_rows, src_cols, dst, dst_cols, irows, orows, wtile, ntl):
                """One (row-block, width-stripe): load, split, 9 matmuls/tile."""
                icols = dst_cols[1] - dst_cols[0] + 2
                xr = xraw.tile([128, STRIPE_IN], f32, tag="xr")
                nc.sync.dma_start(
                    xr[:irows, :icols],
                    src_rows[0][src_rows[1] : src_rows[1] + irows, src_cols : src_cols + icols],
                )
                xh = xhi.tile([128, STRIPE_IN], f32r, tag="xh")
                xl = xlo.tile([128, STRIPE_IN], f32r, tag="xl")
                nc.vector.tensor_copy(xh[:irows, :icols], xr[:irows, :icols])
                nc.vector.tensor_sub(
                    xl[:irows, :icols], xr[:irows, :icols], xh[:irows, :icols]
                )
                yo = yout.tile([126, STRIPE_OW], f32, tag="yo")
                ocols = dst_cols[1] - dst_cols[0]
                for t in range(ntl):
                    pst = psum.tile([126, SLAB_WT], f32, tag="ps")
                    mm = 0
                    for src, base in ((xh, 0), (xl, 0), (xh, 378)):
                        for d in range(3):
                            mm += 1
                            nc.tensor.matmul(
                                pst[:orows, :wtile],
                                wt[:irows, base + d * 126 : base + d * 126 + orows],
                                src[:irows, t * wtile + d : t * wtile + d + wtile],
                                start=(mm == 1),
                                stop=(mm == 9),
                            )
                    nc.scalar.activation(
                        yo[:orows, t * wtile : (t + 1) * wtile],
                        pst[:orows, :wtile],
                        mybir.ActivationFunctionType.Identity,
                        bias=bt[:orows, :],
                        scale=1.0,
                    )
                nc.sync.dma_start(
                    dst[0][dst[1] : dst[1] + orows, dst_cols[0] : dst_cols[1]],
                    yo[:orows, :ocols],
                )

            for j in range(NBLK):
                r0 = j * BLK_OUT
                for h in range(NSTRIPE):
                    c0 = h * STRIPE_OW
                    do_stripe(
                        (xs, r0), c0, (ys, r0), (c0, c0 + STRIPE_OW), 128, BLK_OUT,
                        WT, NWT,
                    )
            # leftover slab: this core's width segment
            do_stripe((xs2, 0), 0, (ys2, 0), (0, SLAB_OC), 128, BLK_OUT, SLAB_WT, SLAB_NT)
    nc.compile()
    return nc


def _get_nc():
    if "nc" not in _cache:
        _cache["nc"] = _build()
    return _cache["nc"]


def _trunc12(a):
    """Truncate fp32 toward zero to 12 significand bits (11 explicit)."""
    return (np.asarray(a, np.float32).view(np.int32) & np.int32(-4096)).view(np.float32)


def make_inputs(x, weight, bias):
    """Host-side shard/prep: per-core input maps for run_bass_kernel_spmd."""
    x = np.asarray(x, np.float32)
    w = np.asarray(weight, np.float32)
    wh = _trunc12(w)
    wl = _trunc12(w - wh)  # w - wh is exact in fp32 and fits 12 bits
    wbm = np.zeros((128, 756), np.float32)
    o = np.arange(BLK_OUT)
    for d in range(3):
        for k in range(3):
            wbm[o + k, d * BLK_OUT + o] = wh[k, d]
            wbm[o + k, 378 + d * BLK_OUT + o] = wl[k, d]
    bcm = np.full((128, 1), np.float32(np.asarray(bias).reshape(-1)[0]), np.float32)
    in_maps = []
    for i in range(NCORES):
        xs2 = np.zeros((128, SLAB_IC), np.float32)
        c0 = i * SLAB_OC
        c1 = min(c0 + SLAB_IC, W)
        xs2[:, : c1 - c0] = x[SLAB_R0 : SLAB_R0 + 128, c0:c1]
        in_maps.append(
            {
                "xs": x[i * RPC : i * RPC + IN_ROWS],
                "xs2": xs2,
                "wb": wbm,
                "bc": bcm,
            }
        )
    return in_maps


def kernel(x, weight, bias):
    from concourse.bass_utils import run_bass_kernel_spmd

    nc = _get_nc()
    in_maps = make_inputs(x, weight, bias)
    res = run_bass_kernel_spmd(nc, in_maps, list(range(NCORES)))
    out = np.empty((OH, OW), np.float32)
    for i in range(NCORES):
        out[i * RPC : (i + 1) * RPC] = res.results[i]["ys"]
        c0 = i * SLAB_OC
        c1 = min(c0 + SLAB_OC, OW)
        out[SLAB_R0:OH, c0:c1] = res.results[i]["ys2"][:, : c1 - c0]
    return out
